# Initial kernel scaffold
#
"""Multi-head self-attention forward on 8 Trainium2 NeuronCores.

Problem: x[4, 2048, 1024] fp32, weights wq/wk/wv/wo [1024, 1024].
  Q,K,V = x @ w{q,k,v}.T (16 heads x 64); causal softmax(QK^T/8)V; out @ wo.T.

Sharding (single SPMD program, per-core data differs only):
  core c: batch b = c//2, head-half hh = c%2 (heads hh*8..hh*8+8),
  wo-half hh (output dims hh*512..). Per pair (2b, 2b+1):
    - each core: Q/K/V projections for its 8 heads (full 2048 tokens),
      causal flash attention for those heads, producing O^T [512, 2048]
    - pairwise AllGather of O^T -> O_full^T [1024, 2048]
    - each core: out-proj against its 512 output dims -> z [2048, 512]
  Host unshard: out[b][:, hh*512:] = core(2b+hh) output.

v4 design (bf16 everywhere; fp8 e4m3 measured 3e-2 max-rel-err per
stage vs the 2e-2 budget, so it is out):
  - x^T / W^T transposed on the HOST, plain contiguous DMAs split across
    the two HWDGE queues (sync + scalar).
  - Attention processes head pairs (2p, 2p+1) as row-tiled concurrent
    K=64 QK matmuls on partition halves (measured exactly 2x).
  - ACT exp is the attention-phase bottleneck ((N+352)/1.2 ns per op).
    Every non-attention matmul is PACED as filler between attention
    groups so the in-order PE queue never idles long enough to drop the
    HAM clock gate: pair0 <- V-proj r8..15 + QK-proj chunk1,
    pair1 <- chunk2, pair2 <- chunk3, pair3 <- out-proj over the six
    early dl-chunks. Lead-in does V r0..7 + chunk0 (ACT does those
    PSUM evacuations while it is otherwise idle).
  - Softmax divide: psum_o is evacuated immediately into packed
    [O_A;O_B] / [den_A;den_B] tiles (4x [64,512] DVE copies) so the apo
    slots free early; then one reciprocal (free-dim-bound) + one
    [128,512] multiply produce both heads' normalized O^T.
  - Chunk 3 is exchanged in two 1024-col halves fired after qb1/qb3 of
    pair 3, so the tail only waits on the second half's collective
    (collective latency ~10us hides under the last attention groups).
"""

import sys

sys.path.insert(0, "/opt/trn_rl_repo")

import ml_dtypes
import numpy as np
import concourse.bass as bass
import concourse.mybir as mybir
import concourse.tile as tile
from concourse import bacc
from concourse.bass_utils import run_bass_kernel_spmd

F32 = mybir.dt.float32
BF16 = mybir.dt.bfloat16
AF = mybir.ActivationFunctionType
OP = mybir.AluOpType

N_CORES = 8
S = 2048          # sequence length
D = 1024          # model dim
HL = 8            # heads per core
DK = 64           # head dim
DL = HL * DK      # local head dims = 512
NEG = -1e30
N_ATTN_GROUPS = 2 + 4 + 6 + 8   # GK=2 groups per head pair

_NC_CACHE = {}


def build():
    nc = bacc.Bacc("TRN2", target_bir_lowering=False, debug=False, num_devices=N_CORES)

    xTb = nc.dram_tensor("xTb", [128, 8, S], BF16, kind="ExternalInput")
    wqb = nc.dram_tensor("wqb", [128, 8, DL], BF16, kind="ExternalInput")
    wkb = nc.dram_tensor("wkb", [128, 8, DL], BF16, kind="ExternalInput")
    wvb = nc.dram_tensor("wvb", [128, 8, DL], BF16, kind="ExternalInput")
    wob = nc.dram_tensor("wob", [128, 8, DL], BF16, kind="ExternalInput")
    idb = nc.dram_tensor("idb", [128, 128], BF16, kind="ExternalInput")
    mkb = nc.dram_tensor("mkb", [128, 128], BF16, kind="ExternalInput")
    z = nc.dram_tensor("z", [S, DL], BF16, kind="ExternalOutput")

    with tile.TileContext(nc) as tc:
        with (
            tc.tile_pool(name="cst", bufs=1) as cst,
            tc.tile_pool(name="per", bufs=1) as per,
            tc.tile_pool(name="dram", bufs=1, space="DRAM") as dram,
            tc.tile_pool(name="ptp", bufs=4) as ptp,
            tc.tile_pool(name="dvp", bufs=3) as dvp,
            tc.tile_pool(name="pps", bufs=2, space="PSUM") as pps,
            tc.tile_pool(name="aps", bufs=2, space="PSUM") as aps,
            tc.tile_pool(name="apo", bufs=2, space="PSUM") as apo,
        ):
            # ---------- constants (host-provided; device-side identity
            # generation measured +14us of startup latency) ----------
            identb = cst.tile([128, 128], BF16)
            nc.sync.dma_start(identb[:], idb[:])
            mtb = cst.tile([128, 128], BF16)
            nc.scalar.dma_start(mtb[:], mkb[:])

            # persistent tiles
            QT = per.tile([128, 4, S], BF16)     # [p(2 heads), chunk, tok]
            KT = per.tile([128, 4, S], BF16)
            # Vaug: [p(tok within kc), head, kc, 0:64]=V, [.., 64:128]=ones
            VA = per.tile([128, HL, 16, 128], BF16)
            OT = per.tile([128, 4, S], BF16)     # [p(dl in chunk), chunk, q]
            woT = per.tile([128, 8, DL], BF16)   # [p(din chunk), chunk, dout]
            nc.gpsimd.memset(VA[:, :, :, DK:128], 1.0)

            cins, gouts = [], []
            for j in range(3):
                cin_t = dram.tile([128, S], BF16, tag=f"cin{j}")
                gout_t = dram.tile([256, S], BF16, tag=f"gout{j}")
                cins.append(cin_t)
                gouts.append(gout_t)
            # chunk 3 exchanged in four 512-col pieces (contiguous tiles)
            cin3p, gout3p = [], []
            for hb in range(4):
                c3_t = dram.tile([128, 512], BF16, tag=f"cin3_{hb}")
                g3_t = dram.tile([256, 512], BF16, tag=f"gout3_{hb}")
                cin3p.append(c3_t)
                gout3p.append(g3_t)

            def emit_attn_pair(p, filler, qb_hook=None, qb_order=(0, 1, 2, 3)):
                """Attention for heads (2p, 2p+1); filler = paced PE work."""
                fill_acc, fill_idx = 0.0, 0
                fill_per = len(filler) / N_ATTN_GROUPS
                for qb in qb_order:
                    q0 = qb * 512
                    nkc = 4 * (qb + 1)
                    poA = apo.tile([128, 512], F32, tag="po")
                    poB = apo.tile([128, 512], F32, tag="po")
                    for g0 in range(0, nkc, 2):
                        kcs = (g0, g0 + 1)
                        psA = aps.tile([128, 1024], F32, tag="ps")
                        psB = aps.tile([128, 1024], F32, tag="ps")
                        # seeds first (K=128, serialize), then paired K=64
                        # QK matmuls (head A rows 0:64, head B rows 64:128)
                        for off, kc in zip((0, 512), kcs):
                            if kc * 128 >= q0:
                                nc.tensor.matmul(
                                    psA[:, off:off + 128], identb[:],
                                    mtb[:], start=True, stop=False)
                                nc.tensor.matmul(
                                    psB[:, off:off + 128], identb[:],
                                    mtb[:], start=True, stop=False)
                        for off, kc in zip((0, 512), kcs):
                            ws = max(0, kc * 128 - q0)
                            diag = kc * 128 >= q0
                            for ps, base in ((psA, 0), (psB, 64)):
                                nc.tensor.matmul(
                                    ps[:, off:off + 512 - ws],
                                    KT[base:base + 64, p,
                                       kc * 128:(kc + 1) * 128],
                                    QT[base:base + 64, p, q0 + ws:q0 + 512],
                                    start=not diag, stop=True)
                        # exp (free affine folds the 1/8 scale)
                        o = 512 + 512 - max(0, kcs[1] * 128 - q0)
                        ptA = ptp.tile([128, 1024], BF16, tag="pt")
                        ptB = ptp.tile([128, 1024], BF16, tag="pt")
                        nc.scalar.activation(ptA[:, 0:o], psA[:, 0:o],
                                             AF.Exp, scale=0.125)
                        nc.scalar.activation(ptB[:, 0:o], psB[:, 0:o],
                                             AF.Exp, scale=0.125)
                        # PV
                        for h, pt, po_ in ((2 * p, ptA, poA),
                                           (2 * p + 1, ptB, poB)):
                            for off, kc in zip((0, 512), kcs):
                                ws = max(0, kc * 128 - q0)
                                nc.tensor.matmul(
                                    po_[:, ws:512],
                                    VA[:, h, kc, :],
                                    pt[:, off:off + 512 - ws],
                                    start=(kc == 0), stop=(kc == nkc - 1))
                        # paced PE filler (proj / out-proj)
                        fill_acc += fill_per
                        while fill_idx < min(fill_acc, len(filler)):
                            filler[fill_idx]()
                            fill_idx += 1
                    # softmax divide: evacuate psum_o right away (packed
                    # layout, frees the apo slots), then one recip + one TT
                    osO = dvp.tile([128, 512], F32, tag="oo")
                    dpack = dvp.tile([128, 512], F32, tag="dp")
                    nc.vector.tensor_copy(dpack[0:64, :], poA[64:128, :])
                    nc.vector.tensor_copy(dpack[64:128, :], poB[64:128, :])
                    nc.vector.tensor_copy(osO[0:64, :], poA[0:64, :])
                    nc.vector.tensor_copy(osO[64:128, :], poB[0:64, :])
                    rec = dvp.tile([128, 512], F32, tag="rc")
                    nc.vector.reciprocal(rec[:], dpack[:])
                    nc.vector.tensor_tensor(
                        OT[:, p, q0:q0 + 512], osO[:], rec[:], OP.mult)
                    if qb_hook is not None:
                        qb_hook(qb)
                while fill_idx < len(filler):
                    filler[fill_idx]()
                    fill_idx += 1

            def exchange_whole(p):
                nc.sync.dma_start(cins[p][:], OT[:, p, :])
                nc.gpsimd.collective_compute(
                    "AllGather", OP.bypass,
                    replica_groups=[[0, 1], [2, 3], [4, 5], [6, 7]],
                    ins=[cins[p][:]], outs=[gouts[p][:]])

            with tc.tile_pool(name="xw", bufs=1) as xw:
                # ---- input DMAs (host pre-transposed bf16) ----
                wTv = xw.tile([128, 8, DL], BF16)
                wTk = xw.tile([128, 8, DL], BF16)
                wTq = xw.tile([128, 8, DL], BF16)
                xT = xw.tile([128, 8, S], BF16)   # [p, din chunk, tok]
                # split input loads across the two HWDGE queues
                nc.sync.dma_start(wTv[:], wvb[:])
                nc.scalar.dma_start(xT[:, 0:2, :], xTb[:, 0:2, :])
                nc.sync.dma_start(xT[:, 2:4, :], xTb[:, 2:4, :])
                nc.scalar.dma_start(xT[:, 4:6, :], xTb[:, 4:6, :])
                nc.sync.dma_start(xT[:, 6:8, :], xTb[:, 6:8, :])
                nc.scalar.dma_start(wTk[:], wkb[:])
                nc.sync.dma_start(wTq[:], wqb[:])
                nc.scalar.dma_start(woT[:], wob[:])

                # PE warmup: keep the HAM clock-gate busy during the DMA
                # ramp so the first projection matmuls run at 2.4 GHz
                wrm = aps.tile([128, 1024], F32, tag="ps")
                for i in range(72):
                    nc.tensor.matmul(wrm[:, 0:128], identb[:], identb[:],
                                     start=True, stop=True)

                def v_group(r, evac_eng):
                    def emit(r=r, evac_eng=evac_eng):
                        pp = pps.tile([128, 512], F32, tag="pp")
                        for i in range(8):
                            nc.tensor.matmul(
                                pp[:],
                                xT[:, i, r * 128:(r + 1) * 128],
                                wTv[:, i, :],
                                start=(i == 0), stop=(i == 7))
                        # strided interleave into VA
                        if evac_eng == "act":
                            nc.scalar.activation(VA[:, :, r, 0:DK], pp[:],
                                                 AF.Copy)
                        else:
                            nc.vector.tensor_copy(VA[:, :, r, 0:DK], pp[:])
                    return emit

                def qk_chunk_groups(c, evac_eng):
                    """Per-(mat, tb) emission closures for proj chunk c."""
                    out = []
                    for wT, dst in ((wTk, KT), (wTq, QT)):
                        for tb in range(4):
                            def emit(wT=wT, dst=dst, tb=tb):
                                pp = pps.tile([128, 512], F32, tag="pp")
                                for i in range(8):
                                    nc.tensor.matmul(
                                        pp[:],
                                        wT[:, i, c * 128:(c + 1) * 128],
                                        xT[:, i, tb * 512:(tb + 1) * 512],
                                        start=(i == 0), stop=(i == 7))
                                if evac_eng == "act":
                                    nc.scalar.activation(
                                        dst[:, c, tb * 512:(tb + 1) * 512],
                                        pp[:], AF.Copy)
                                else:
                                    nc.vector.tensor_copy(
                                        dst[:, c, tb * 512:(tb + 1) * 512],
                                        pp[:])
                            out.append(emit)
                    return out

                # ---- lead-in: V r0..7 + QK chunk 0 (ACT evacs; ACT idle) --
                for r in range(8):
                    v_group(r, "act")()
                for emit in qk_chunk_groups(0, "act"):
                    emit()

                # ---- pairs 0..2 ----
                # pair0 filler: V r8..15 (needed by pair0 qb3!) then chunk1
                f0 = [v_group(r, "dve") for r in range(8, 16)] \
                    + qk_chunk_groups(1, "dve")
                emit_attn_pair(0, f0)
                exchange_whole(0)
                emit_attn_pair(1, qk_chunk_groups(2, "dve"))
                exchange_whole(1)
                emit_attn_pair(2, qk_chunk_groups(3, "dve"))
                exchange_whole(2)

            with tc.tile_pool(name="outp", bufs=1) as outp:
                # ---- pair 3: halved exchange + out-proj filler ----
                otf = outp.tile([128, 8, S], BF16)
                zt1s = {}
                g1js = (0, 1, 2, 4, 5, 6)

                def g1_emit(qt):
                    if qt in zt1s:
                        return
                    pz = pps.tile([128, DL], F32, tag="pp")
                    for n, j in enumerate(g1js):
                        nc.tensor.matmul(
                            pz[:],
                            otf[:, j, qt * 128:(qt + 1) * 128],
                            woT[:, j, :],
                            start=(n == 0), stop=(n == len(g1js) - 1))
                    zt1 = outp.tile([128, DL], F32, tag=f"zt1_{qt}")
                    nc.vector.tensor_copy(zt1[:], pz[:])
                    zt1s[qt] = zt1

                def outproj_g1_work():
                    work = []
                    for j in g1js:
                        def load(j=j):
                            src, row = ((gouts[j], 0) if j < 4 else
                                        (gouts[j - 4], 128))
                            nc.sync.dma_start(otf[:, j, :],
                                              src[row:row + 128, :])
                        work.append(load)
                    # qt order matches the reversed qb processing of pair 3
                    for qb in (3, 2, 1, 0):
                        for qt in range(4 * qb, 4 * qb + 4):
                            work.append(lambda qt=qt: g1_emit(qt))
                    return work

                def g2_piece(pb, dma_eng=None):
                    """Final out-proj (chunks 3,7) for query block pb."""
                    eng = dma_eng if dma_eng is not None else nc.sync
                    q0 = pb * 512
                    for jj, row in ((3, 0), (7, 128)):
                        eng.dma_start(otf[:, jj, q0:q0 + 512],
                                      gout3p[pb][row:row + 128, :])
                    for qt in range(4 * pb, 4 * pb + 4):
                        g1_emit(qt)
                        pz = pps.tile([128, DL], F32, tag="pp")
                        for n, j in enumerate((3, 7)):
                            nc.tensor.matmul(
                                pz[:],
                                otf[:, j, qt * 128:(qt + 1) * 128],
                                woT[:, j, :],
                                start=(n == 0), stop=(n == 1))
                        zt = dvp.tile([128, DL], BF16, tag="zt")
                        nc.vector.tensor_tensor(zt[:], pz[:], zt1s[qt][:],
                                                OP.add)
                        eng.dma_start(z[qt * 128:(qt + 1) * 128, :],
                                      zt[:])

                # pair 3 runs qb3 first so the big query blocks exchange
                # early; each qb's 512-col piece goes out as soon as its
                # divide lands, and its out-proj runs two steps later
                qb_order = (3, 2, 1, 0)

                def qb_hook(qb):
                    nc.sync.dma_start(cin3p[qb][:],
                                      OT[:, 3, qb * 512:qb * 512 + 512])
                    nc.gpsimd.collective_compute(
                        "AllGather", OP.bypass,
                        replica_groups=[[0, 1], [2, 3], [4, 5], [6, 7]],
                        ins=[cin3p[qb][:]], outs=[gout3p[qb][:]])
                    step = qb_order.index(qb)
                    if step >= 2:
                        g2_piece(qb_order[step - 2])

                emit_attn_pair(3, outproj_g1_work(), qb_hook=qb_hook,
                               qb_order=qb_order)
                # tail: ACT is done with exp by now - use its HWDGE queue
                g2_piece(qb_order[2], dma_eng=nc.scalar)
                g2_piece(qb_order[3], dma_eng=nc.scalar)

    nc.compile()
    return nc


def _get_nc():
    if "nc" not in _NC_CACHE:
        _NC_CACHE["nc"] = build()
    return _NC_CACHE["nc"]


def kernel(x, wq, wk, wv, wo, _trace=False):
    bf = ml_dtypes.bfloat16
    x = np.asarray(x, dtype=np.float32)
    b, s, d = x.shape
    assert (b, s, d) == (4, S, D)

    def wprep(w, hh):
        wh = np.asarray(w, dtype=np.float32)[hh * DL:(hh + 1) * DL, :]
        # [DL, D] -> transposed [D, DL] -> [128, 8, DL]
        return np.ascontiguousarray(
            wh.T.reshape(8, 128, DL).transpose(1, 0, 2)).astype(bf)

    idb_h = np.eye(128, dtype=np.float32).astype(bf)
    mkb_h = np.where(np.arange(128)[:, None] > np.arange(128)[None, :],
                     np.float32(NEG), np.float32(0.0)).astype(bf)

    xTs = []
    for bi in range(4):
        xT = np.ascontiguousarray(
            x[bi].T.reshape(8, 128, S).transpose(1, 0, 2)).astype(bf)
        xTs.append(xT)

    in_maps = []
    for c in range(N_CORES):
        bi, hh = c // 2, c % 2
        in_maps.append({
            "xTb": xTs[bi],
            "wqb": wprep(wq, hh),
            "wkb": wprep(wk, hh),
            "wvb": wprep(wv, hh),
            "wob": wprep(wo, hh),
            "idb": idb_h,
            "mkb": mkb_h,
        })

    nc = _get_nc()
    res = run_bass_kernel_spmd(nc, in_maps, core_ids=list(range(N_CORES)),
                               trace=_trace)

    out = np.empty((4, S, D), dtype=np.float32)
    for c in range(N_CORES):
        bi, hh = c // 2, c % 2
        out[bi][:, hh * DL:(hh + 1) * DL] = res.results[c]["z"].astype(
            np.float32)
    if _trace:
        kernel.last_exec_time_ns = res.exec_time_ns
    return out



# revision 1
# speedup vs baseline: 1.1779x; 1.1779x over previous
"""Multi-head self-attention forward on 8 Trainium2 NeuronCores.

Problem: x[4, 2048, 1024] fp32, weights wq/wk/wv/wo [1024, 1024].
  Q,K,V = x @ w{q,k,v}.T (16 heads x 64); causal softmax(QK^T/8)V; out @ wo.T.

Sharding (single SPMD program, per-core data differs only):
  core c: batch b = c//2, head-half hh = c%2 (heads hh*8..hh*8+8),
  wo-half hh (output dims hh*512..). Per pair (2b, 2b+1):
    - each core: Q/K/V projections for its 8 heads (full 2048 tokens),
      causal flash attention for those heads, producing O^T [512, 2048]
    - pairwise AllGather of O^T -> O_full^T [1024, 2048]
    - each core: out-proj against its 512 output dims -> z [2048, 512]
  Host unshard: out[b][:, hh*512:] = core(2b+hh) output.

v4 design (bf16 everywhere; fp8 e4m3 measured 3e-2 max-rel-err per
stage vs the 2e-2 budget, so it is out):
  - x^T / W^T transposed on the HOST, plain contiguous DMAs split across
    the two HWDGE queues (sync + scalar).
  - Attention processes head pairs (2p, 2p+1) as row-tiled concurrent
    K=64 QK matmuls on partition halves (measured exactly 2x).
  - ACT exp is the attention-phase bottleneck ((N+352)/1.2 ns per op).
    Every non-attention matmul is PACED as filler between attention
    groups so the in-order PE queue never idles long enough to drop the
    HAM clock gate: pair0 <- V-proj r8..15 + QK-proj chunk1,
    pair1 <- chunk2, pair2 <- chunk3, pair3 <- out-proj over the six
    early dl-chunks. Lead-in does V r0..7 + chunk0 (ACT does those
    PSUM evacuations while it is otherwise idle).
  - Softmax divide: psum_o is evacuated immediately into packed
    [O_A;O_B] / [den_A;den_B] tiles (4x [64,512] DVE copies) so the apo
    slots free early; then one reciprocal (free-dim-bound) + one
    [128,512] multiply produce both heads' normalized O^T.
  - Chunk 3 is exchanged in two 1024-col halves fired after qb1/qb3 of
    pair 3, so the tail only waits on the second half's collective
    (collective latency ~10us hides under the last attention groups).
"""

import sys

sys.path.insert(0, "/opt/trn_rl_repo")

import ml_dtypes
import numpy as np
import concourse.bass as bass
import concourse.mybir as mybir
import concourse.tile as tile
from concourse import bacc
from concourse.bass_utils import run_bass_kernel_spmd

F32 = mybir.dt.float32
BF16 = mybir.dt.bfloat16
AF = mybir.ActivationFunctionType
OP = mybir.AluOpType

N_CORES = 8
S = 2048          # sequence length
D = 1024          # model dim
HL = 8            # heads per core
DK = 64           # head dim
DL = HL * DK      # local head dims = 512
NEG = -1e30
N_ATTN_GROUPS = 2 + 4 + 6 + 8   # GK=2 groups per head pair

_NC_CACHE = {}


def build():
    nc = bacc.Bacc("TRN2", target_bir_lowering=False, debug=False, num_devices=N_CORES)

    xTb = nc.dram_tensor("xTb", [128, 8, S], BF16, kind="ExternalInput")
    wqb = nc.dram_tensor("wqb", [128, 8, DL], BF16, kind="ExternalInput")
    wkb = nc.dram_tensor("wkb", [128, 8, DL], BF16, kind="ExternalInput")
    wvb = nc.dram_tensor("wvb", [128, 8, DL], BF16, kind="ExternalInput")
    wob = nc.dram_tensor("wob", [128, 8, DL], BF16, kind="ExternalInput")
    idb = nc.dram_tensor("idb", [128, 128], BF16, kind="ExternalInput")
    mkb = nc.dram_tensor("mkb", [128, 128], BF16, kind="ExternalInput")
    z = nc.dram_tensor("z", [S, DL], BF16, kind="ExternalOutput")

    with tile.TileContext(nc) as tc:
        with (
            tc.tile_pool(name="cst", bufs=1) as cst,
            tc.tile_pool(name="per", bufs=1) as per,
            tc.tile_pool(name="dram", bufs=1, space="DRAM") as dram,
            tc.tile_pool(name="ptp", bufs=4) as ptp,
            tc.tile_pool(name="dvp", bufs=3) as dvp,
            tc.tile_pool(name="pps", bufs=2, space="PSUM") as pps,
            tc.tile_pool(name="aps", bufs=2, space="PSUM") as aps,
            tc.tile_pool(name="apo", bufs=2, space="PSUM") as apo,
        ):
            # ---------- constants (host-provided; device-side identity
            # generation measured +14us of startup latency) ----------
            identb = cst.tile([128, 128], BF16)
            nc.sync.dma_start(identb[:], idb[:])
            mtb = cst.tile([128, 128], BF16)
            nc.scalar.dma_start(mtb[:], mkb[:])

            # persistent tiles
            QT = per.tile([128, 4, S], BF16)     # [p(2 heads), chunk, tok]
            KT = per.tile([128, 4, S], BF16)
            # Vaug: [p(tok within kc), head, kc, 0:64]=V, [.., 64:128]=ones
            VA = per.tile([128, HL, 16, 128], BF16)
            OT = per.tile([128, 4, S], BF16)     # [p(dl in chunk), chunk, q]
            woT = per.tile([128, 8, DL], BF16)   # [p(din chunk), chunk, dout]
            nc.gpsimd.memset(VA[:, :, :, DK:128], 1.0)

            cins, gouts = [], []
            for j in range(3):
                cin_t = dram.tile([128, S], BF16, tag=f"cin{j}")
                gout_t = dram.tile([256, S], BF16, tag=f"gout{j}")
                cins.append(cin_t)
                gouts.append(gout_t)
            # chunk 3 exchanged in four 512-col pieces (contiguous tiles)
            cin3p, gout3p = [], []
            for hb in range(4):
                c3_t = dram.tile([128, 512], BF16, tag=f"cin3_{hb}")
                g3_t = dram.tile([256, 512], BF16, tag=f"gout3_{hb}")
                cin3p.append(c3_t)
                gout3p.append(g3_t)

            def emit_attn_pair(p, filler, qb_hook=None, qb_order=(0, 1, 2, 3)):
                """Attention for heads (2p, 2p+1); filler = paced PE work."""
                fill_acc, fill_idx = 0.0, 0
                fill_per = len(filler) / N_ATTN_GROUPS
                for qb in qb_order:
                    q0 = qb * 512
                    nkc = 4 * (qb + 1)
                    poA = apo.tile([128, 512], F32, tag="po")
                    poB = apo.tile([128, 512], F32, tag="po")
                    for g0 in range(0, nkc, 2):
                        kcs = (g0, g0 + 1)
                        psA = aps.tile([128, 1024], F32, tag="ps")
                        psB = aps.tile([128, 1024], F32, tag="ps")
                        # seeds first (K=128, serialize), then paired K=64
                        # QK matmuls (head A rows 0:64, head B rows 64:128)
                        for off, kc in zip((0, 512), kcs):
                            if kc * 128 >= q0:
                                nc.tensor.matmul(
                                    psA[:, off:off + 128], identb[:],
                                    mtb[:], start=True, stop=False)
                                nc.tensor.matmul(
                                    psB[:, off:off + 128], identb[:],
                                    mtb[:], start=True, stop=False)
                        for off, kc in zip((0, 512), kcs):
                            ws = max(0, kc * 128 - q0)
                            diag = kc * 128 >= q0
                            for ps, base in ((psA, 0), (psB, 64)):
                                nc.tensor.matmul(
                                    ps[:, off:off + 512 - ws],
                                    KT[base:base + 64, p,
                                       kc * 128:(kc + 1) * 128],
                                    QT[base:base + 64, p, q0 + ws:q0 + 512],
                                    start=not diag, stop=True)
                        # exp (free affine folds the 1/8 scale)
                        o = 512 + 512 - max(0, kcs[1] * 128 - q0)
                        ptA = ptp.tile([128, 1024], BF16, tag="pt")
                        ptB = ptp.tile([128, 1024], BF16, tag="pt")
                        nc.scalar.activation(ptA[:, 0:o], psA[:, 0:o],
                                             AF.Exp, scale=0.125)
                        nc.scalar.activation(ptB[:, 0:o], psB[:, 0:o],
                                             AF.Exp, scale=0.125)
                        # PV
                        for h, pt, po_ in ((2 * p, ptA, poA),
                                           (2 * p + 1, ptB, poB)):
                            for off, kc in zip((0, 512), kcs):
                                ws = max(0, kc * 128 - q0)
                                nc.tensor.matmul(
                                    po_[:, ws:512],
                                    VA[:, h, kc, :],
                                    pt[:, off:off + 512 - ws],
                                    start=(kc == 0), stop=(kc == nkc - 1))
                        # paced PE filler (proj / out-proj)
                        fill_acc += fill_per
                        while fill_idx < min(fill_acc, len(filler)):
                            filler[fill_idx]()
                            fill_idx += 1
                    # softmax divide: evacuate psum_o right away (packed
                    # layout, frees the apo slots), then one recip + one TT
                    osO = dvp.tile([128, 512], F32, tag="oo")
                    dpack = dvp.tile([128, 512], F32, tag="dp")
                    nc.vector.tensor_copy(dpack[0:64, :], poA[64:128, :])
                    nc.vector.tensor_copy(dpack[64:128, :], poB[64:128, :])
                    nc.vector.tensor_copy(osO[0:64, :], poA[0:64, :])
                    nc.vector.tensor_copy(osO[64:128, :], poB[0:64, :])
                    rec = dvp.tile([128, 512], F32, tag="rc")
                    nc.vector.reciprocal(rec[:], dpack[:])
                    nc.vector.tensor_tensor(
                        OT[:, p, q0:q0 + 512], osO[:], rec[:], OP.mult)
                    if qb_hook is not None:
                        qb_hook(qb)
                while fill_idx < len(filler):
                    filler[fill_idx]()
                    fill_idx += 1

            def exchange_whole(p):
                nc.sync.dma_start(cins[p][:], OT[:, p, :])
                nc.gpsimd.collective_compute(
                    "AllGather", OP.bypass,
                    replica_groups=[[0, 1], [2, 3], [4, 5], [6, 7]],
                    ins=[cins[p][:]], outs=[gouts[p][:]])

            with tc.tile_pool(name="xw", bufs=1) as xw:
                # ---- input DMAs (host pre-transposed bf16) ----
                wTv = xw.tile([128, 8, DL], BF16)
                wTk = xw.tile([128, 8, DL], BF16)
                wTq = xw.tile([128, 8, DL], BF16)
                xT = xw.tile([128, 8, S], BF16)   # [p, din chunk, tok]
                # split input loads across the two HWDGE queues
                nc.sync.dma_start(wTv[:], wvb[:])
                nc.scalar.dma_start(xT[:, 0:2, :], xTb[:, 0:2, :])
                nc.sync.dma_start(xT[:, 2:4, :], xTb[:, 2:4, :])
                nc.scalar.dma_start(xT[:, 4:6, :], xTb[:, 4:6, :])
                nc.sync.dma_start(xT[:, 6:8, :], xTb[:, 6:8, :])
                nc.scalar.dma_start(wTk[:], wkb[:])
                nc.sync.dma_start(wTq[:], wqb[:])
                nc.scalar.dma_start(woT[:], wob[:])

                # PE warmup: keep the HAM clock-gate busy during the DMA
                # ramp so the first projection matmuls run at 2.4 GHz
                wrm = aps.tile([128, 1024], F32, tag="ps")
                for i in range(72):
                    nc.tensor.matmul(wrm[:, 0:128], identb[:], identb[:],
                                     start=True, stop=True)

                def v_group(r, evac_eng):
                    def emit(r=r, evac_eng=evac_eng):
                        pp = pps.tile([128, 512], F32, tag="pp")
                        for i in range(8):
                            nc.tensor.matmul(
                                pp[:],
                                xT[:, i, r * 128:(r + 1) * 128],
                                wTv[:, i, :],
                                start=(i == 0), stop=(i == 7))
                        # strided interleave into VA
                        if evac_eng == "act":
                            nc.scalar.activation(VA[:, :, r, 0:DK], pp[:],
                                                 AF.Copy)
                        else:
                            nc.vector.tensor_copy(VA[:, :, r, 0:DK], pp[:])
                    return emit

                def qk_chunk_groups(c, evac_eng):
                    """Per-(mat, tb) emission closures for proj chunk c."""
                    out = []
                    for wT, dst in ((wTk, KT), (wTq, QT)):
                        for tb in range(4):
                            def emit(wT=wT, dst=dst, tb=tb):
                                pp = pps.tile([128, 512], F32, tag="pp")
                                for i in range(8):
                                    nc.tensor.matmul(
                                        pp[:],
                                        wT[:, i, c * 128:(c + 1) * 128],
                                        xT[:, i, tb * 512:(tb + 1) * 512],
                                        start=(i == 0), stop=(i == 7))
                                if evac_eng == "act":
                                    nc.scalar.activation(
                                        dst[:, c, tb * 512:(tb + 1) * 512],
                                        pp[:], AF.Copy)
                                else:
                                    nc.vector.tensor_copy(
                                        dst[:, c, tb * 512:(tb + 1) * 512],
                                        pp[:])
                            out.append(emit)
                    return out

                # ---- lead-in: V r0..7 + QK chunk 0 (ACT evacs; ACT idle) --
                for r in range(8):
                    v_group(r, "act")()
                for emit in qk_chunk_groups(0, "act"):
                    emit()

                # ---- pairs 0..2 ----
                # pair0 filler: V r8..15 (needed by pair0 qb3!) then chunk1
                f0 = [v_group(r, "dve") for r in range(8, 16)] \
                    + qk_chunk_groups(1, "dve")
                emit_attn_pair(0, f0)
                exchange_whole(0)
                emit_attn_pair(1, qk_chunk_groups(2, "dve"))
                exchange_whole(1)
                emit_attn_pair(2, qk_chunk_groups(3, "dve"))
                exchange_whole(2)

            with tc.tile_pool(name="outp", bufs=1) as outp:
                # ---- pair 3: halved exchange + out-proj filler ----
                otf = outp.tile([128, 8, S], BF16)
                zt1s = {}
                g1js = (0, 1, 2, 4, 5, 6)

                def g1_emit(qt):
                    if qt in zt1s:
                        return
                    pz = pps.tile([128, DL], F32, tag="pp")
                    for n, j in enumerate(g1js):
                        nc.tensor.matmul(
                            pz[:],
                            otf[:, j, qt * 128:(qt + 1) * 128],
                            woT[:, j, :],
                            start=(n == 0), stop=(n == len(g1js) - 1))
                    zt1 = outp.tile([128, DL], F32, tag=f"zt1_{qt}")
                    nc.vector.tensor_copy(zt1[:], pz[:])
                    zt1s[qt] = zt1

                def outproj_g1_work():
                    work = []
                    for j in g1js:
                        def load(j=j):
                            src, row = ((gouts[j], 0) if j < 4 else
                                        (gouts[j - 4], 128))
                            nc.sync.dma_start(otf[:, j, :],
                                              src[row:row + 128, :])
                        work.append(load)
                    # qt order matches the reversed qb processing of pair 3
                    for qb in (3, 2, 1, 0):
                        for qt in range(4 * qb, 4 * qb + 4):
                            work.append(lambda qt=qt: g1_emit(qt))
                    return work

                def g2_piece(pb, dma_eng=None):
                    """Final out-proj (chunks 3,7) for query block pb."""
                    eng = dma_eng if dma_eng is not None else nc.sync
                    q0 = pb * 512
                    for jj, row in ((3, 0), (7, 128)):
                        eng.dma_start(otf[:, jj, q0:q0 + 512],
                                      gout3p[pb][row:row + 128, :])
                    for qt in range(4 * pb, 4 * pb + 4):
                        g1_emit(qt)
                        pz = pps.tile([128, DL], F32, tag="pp")
                        for n, j in enumerate((3, 7)):
                            nc.tensor.matmul(
                                pz[:],
                                otf[:, j, qt * 128:(qt + 1) * 128],
                                woT[:, j, :],
                                start=(n == 0), stop=(n == 1))
                        zt = dvp.tile([128, DL], BF16, tag="zt")
                        nc.vector.tensor_tensor(zt[:], pz[:], zt1s[qt][:],
                                                OP.add)
                        eng.dma_start(z[qt * 128:(qt + 1) * 128, :],
                                      zt[:])

                # pair 3 runs qb3 first so the big query blocks exchange
                # early; each qb's 512-col piece goes out as soon as its
                # divide lands, and its out-proj runs two steps later
                qb_order = (3, 2, 1, 0)

                def qb_hook(qb):
                    nc.sync.dma_start(cin3p[qb][:],
                                      OT[:, 3, qb * 512:qb * 512 + 512])
                    nc.gpsimd.collective_compute(
                        "AllGather", OP.bypass,
                        replica_groups=[[0, 1], [2, 3], [4, 5], [6, 7]],
                        ins=[cin3p[qb][:]], outs=[gout3p[qb][:]])
                    step = qb_order.index(qb)
                    if step >= 2:
                        g2_piece(qb_order[step - 2])

                emit_attn_pair(3, outproj_g1_work(), qb_hook=qb_hook,
                               qb_order=qb_order)
                # tail: ACT is done with exp by now - use its HWDGE queue
                g2_piece(qb_order[2], dma_eng=nc.scalar)
                g2_piece(qb_order[3], dma_eng=nc.scalar)

    nc.compile()
    return nc


def _get_nc():
    if "nc" not in _NC_CACHE:
        _NC_CACHE["nc"] = build()
    return _NC_CACHE["nc"]


def kernel(x, wq, wk, wv, wo, _trace=False):
    bf = ml_dtypes.bfloat16
    x = np.asarray(x, dtype=np.float32)
    b, s, d = x.shape
    assert (b, s, d) == (4, S, D)

    def wprep(w, hh):
        wh = np.asarray(w, dtype=np.float32)[hh * DL:(hh + 1) * DL, :]
        # [DL, D] -> transposed [D, DL] -> [128, 8, DL]
        return np.ascontiguousarray(
            wh.T.reshape(8, 128, DL).transpose(1, 0, 2)).astype(bf)

    idb_h = np.eye(128, dtype=np.float32).astype(bf)
    mkb_h = np.where(np.arange(128)[:, None] > np.arange(128)[None, :],
                     np.float32(NEG), np.float32(0.0)).astype(bf)

    xTs = []
    for bi in range(4):
        xT = np.ascontiguousarray(
            x[bi].T.reshape(8, 128, S).transpose(1, 0, 2)).astype(bf)
        xTs.append(xT)

    in_maps = []
    for c in range(N_CORES):
        bi, hh = c // 2, c % 2
        in_maps.append({
            "xTb": xTs[bi],
            "wqb": wprep(wq, hh),
            "wkb": wprep(wk, hh),
            "wvb": wprep(wv, hh),
            "wob": wprep(wo, hh),
            "idb": idb_h,
            "mkb": mkb_h,
        })

    nc = _get_nc()
    res = run_bass_kernel_spmd(nc, in_maps, core_ids=list(range(N_CORES)),
                               trace=_trace)

    out = np.empty((4, S, D), dtype=np.float32)
    for c in range(N_CORES):
        bi, hh = c // 2, c % 2
        out[bi][:, hh * DL:(hh + 1) * DL] = res.results[c]["z"].astype(
            np.float32)
    if _trace:
        kernel.last_exec_time_ns = res.exec_time_ns
    return out

